# revision 1
# baseline (speedup 1.0000x reference)
"""MiniBert (embed + LayerNorm + single-head attention) on 8 TRN2 NeuronCores.

Strategy: data-parallel over batch (4 sequences per core), embedding table /
pos table / projection weights replicated to every core.

Per-core pipeline (per sequence):
  gather word_emb rows (indirect DMA) -> += pos_emb (DVE)
  LayerNorm: bn_stats/bn_aggr (DVE), rsqrt as exp(-.5*ln(var+eps)) (ACT),
             normalize via ACT Identity(scale=rs, bias=-mu*rs) -> x (fp32r)
  x^T via PE transpose (fp32r)
  Q^T = Wq'-slices @ x^T, K^T likewise (fp32r, gamma and 1/sqrt(D) folded into
  Wq' on host); V = x^T-slices @ Wv' (stored fp16)
  S = Q^T.T @ K^T in PSUM; row-max (DVE, negated); P = exp(S - max) on ACT
  (fp16) with accumulated row sums l; P^T via DMA transpose (fp16)
  O = P@V accumulated in PSUM; final scale by 1/l (DVE) -> DMA out
"""
import math
import numpy as np

from concourse import bass, mybir
import concourse.tile as tile
from concourse.bass_utils import run_bass_kernel_spmd
from concourse.masks import make_identity

P = 128
D = 512
VOC = 32000
N_CORES = 8

F32 = mybir.dt.float32
F32R = mybir.dt.float32r
F16 = mybir.dt.float16
I32 = mybir.dt.int32

AF = mybir.ActivationFunctionType
ALU = mybir.AluOpType
AX = mybir.AxisListType


def fix_fat_waits(nc, max_waits=1):
    """Walrus rejects instructions carrying more than ~1 semaphore wait. Tile
    occasionally emits joins (notably the kernel-tail drain) with one wait per
    producing processor. Split the extras into a chain of single-wait NoOps on
    the same engine, inserted immediately before the original instruction."""
    n_new = 0
    for bb in nc.main_func.blocks:
        insts = bb.instructions
        i = 0
        while i < len(insts):
            ins = insts[i]
            si = ins.sync_info
            if si and si.on_wait and len(si.on_wait) > max_waits:
                waits = list(si.on_wait)
                keep = waits[-max_waits:]
                extra = waits[:-max_waits]
                ins.sync_info = mybir.SyncInfo(
                    on_wait=keep, on_update=list(si.on_update or []))
                for j, w in enumerate(extra):
                    nop = mybir.InstNoOp(name=f"W-split-{n_new}", ins=[], outs=[])
                    n_new += 1
                    nop.engine = ins.engine
                    nop.sync_info = mybir.SyncInfo(on_wait=[w], on_update=[])
                    insts.insert(i + j, nop)
                i += len(extra)
            i += 1
    return n_new


def build(b_per_core: int, s_len: int, voc: int, apply_beta: bool, debug: bool = False, stages: int = 99,
          bufs_ebuf: int = 2, bufs_sm: int = 3, bufs_psm: int = 2, bufs_pss: int = 3):
    """Build the per-core SPMD program. All cores run this same module."""
    nt = s_len // P          # token tiles per sequence (8)
    dk = D // P              # feature tiles (4)
    nchunk = 2               # moving-dim chunks for N=s_len matmuls (512 each)
    ckw = s_len // nchunk    # 512

    nc = bass.Bass()

    dbg = {}
    if debug:
        dbg["x"] = nc.dram_tensor("dbg_x", [P, nt * D], F32, kind="ExternalOutput")
        dbg["xt"] = nc.dram_tensor("dbg_xt", [P, dk * s_len], F32, kind="ExternalOutput")
        dbg["qt"] = nc.dram_tensor("dbg_qt", [P, dk * s_len], F32, kind="ExternalOutput")
        dbg["kt"] = nc.dram_tensor("dbg_kt", [P, dk * s_len], F32, kind="ExternalOutput")
        dbg["v"] = nc.dram_tensor("dbg_v", [P, nt * D], F32, kind="ExternalOutput")
        dbg["s"] = nc.dram_tensor("dbg_s", [P, s_len], F32, kind="ExternalOutput")
        dbg["p"] = nc.dram_tensor("dbg_p", [P, s_len], F32, kind="ExternalOutput")
        dbg["pt"] = nc.dram_tensor("dbg_pt", [P, nt * P], F32, kind="ExternalOutput")

    ids_d = nc.dram_tensor("ids", [P, b_per_core * nt], I32, kind="ExternalInput")
    wemb_d = nc.dram_tensor("wemb", [voc, D], F32, kind="ExternalInput")
    pos_d = nc.dram_tensor("pos", [s_len, D], F32, kind="ExternalInput")
    wq_d = nc.dram_tensor("wq", [D, D], F32, kind="ExternalInput")
    wk_d = nc.dram_tensor("wk", [D, D], F32, kind="ExternalInput")
    wv_d = nc.dram_tensor("wv", [D, D], F32, kind="ExternalInput")
    if apply_beta:
        beta_d = nc.dram_tensor("beta_b", [P, D], F32, kind="ExternalInput")
    out_d = nc.dram_tensor("out", [b_per_core * s_len, D], F32, kind="ExternalOutput")

    with tile.TileContext(nc) as tc:
        with (
            tc.tile_pool(name="pers", bufs=1) as pers,
            tc.tile_pool(name="ebuf", bufs=bufs_ebuf) as ebuf,
            tc.tile_pool(name="proj", bufs=1) as proj,
            tc.tile_pool(name="sm", bufs=bufs_sm) as sm,
            tc.tile_pool(name="ps_misc", bufs=bufs_psm, space="PSUM") as ps_misc,
            tc.tile_pool(name="ps_s", bufs=bufs_pss, space="PSUM") as ps_s,
        ):
            # ---------------- preamble: persistent tiles ----------------
            ids_t = pers.tile([P, b_per_core * nt], I32, tag="ids")
            nc.sync.dma_start(out=ids_t[:], in_=ids_d[:, :])

            pos_t = pers.tile([P, nt, D], F32, tag="pos")
            nc.sync.dma_start(
                out=pos_t[:],
                in_=pos_d.rearrange("(a p) d -> p a d", p=P),
            )

            w_stage = pers.tile([P, dk, D], F32, tag="wstage")
            w_r = {}
            for name, wd in (("wq", wq_d), ("wk", wk_d), ("wv", wv_d)):
                wt = pers.tile([P, dk, D], F32R, tag=f"{name}r")
                nc.sync.dma_start(
                    out=w_stage[:], in_=wd.rearrange("(a p) n -> p a n", p=P))
                nc.vector.tensor_copy(out=wt[:], in_=w_stage[:])
                w_r[name] = wt

            if apply_beta:
                beta_t = pers.tile([P, D], F32, tag="betab")
                nc.sync.dma_start(out=beta_t[:], in_=beta_d[:, :])

            ident_f = pers.tile([P, P], F32, tag="ident_f")
            make_identity(nc, ident_f[:])
            ident = pers.tile([P, P], F32R, tag="ident")
            nc.vector.tensor_copy(out=ident[:], in_=ident_f[:])

            epsb = pers.tile([P, 1], F32, tag="epsb")
            nc.gpsimd.memset(epsb[:], 1e-5)

            # ---------------- per-sequence pipeline ----------------
            stages_eff = 99 if stages < 0 else stages

            def emit_batch(b):
                if stages_eff < 1:
                    return
                # --- embed + layernorm ---
                e_all = ebuf.tile([P, nt, D], F32, tag="e_all")
                for j in range(nt):
                    nc.gpsimd.indirect_dma_start(
                        out=e_all[:, j, :],
                        out_offset=None,
                        in_=wemb_d[:],
                        in_offset=bass.IndirectOffsetOnAxis(
                            ap=ids_t[:, b * nt + j: b * nt + j + 1], axis=0),
                    )
                if stages_eff < 2:
                    return
                for j in range(nt):
                    nc.vector.tensor_tensor(
                        out=e_all[:, j, :], in0=e_all[:, j, :],
                        in1=pos_t[:, j, :], op=ALU.add)

                s6 = ebuf.tile([P, nt, 6], F32, tag="s6")
                mv = ebuf.tile([P, nt, 2], F32, tag="mv")
                for j in range(nt):
                    nc.vector.bn_stats(out=s6[:, j, :], in_=e_all[:, j, :])
                    nc.vector.bn_aggr(out=mv[:, j, :], in_=s6[:, j, :])

                lnv = ebuf.tile([P, nt], F32, tag="lnv")
                rs = ebuf.tile([P, nt], F32, tag="rs")
                nmurs = ebuf.tile([P, nt], F32, tag="nmurs")
                # rs = exp(-0.5*ln(var + eps)) == rsqrt(var + eps)
                nc.scalar.activation(out=lnv[:], in_=mv[:, :, 1], func=AF.Ln,
                                     bias=epsb[:, 0:1], scale=1.0)
                nc.scalar.activation(out=rs[:], in_=lnv[:], func=AF.Exp,
                                     bias=0.0, scale=-0.5)
                # nmurs = -mean * rs
                nc.vector.tensor_tensor(out=nmurs[:], in0=mv[:, :, 0],
                                        in1=rs[:], op=ALU.mult)
                nc.vector.tensor_scalar_mul(out=nmurs[:], in0=nmurs[:],
                                            scalar1=-1.0)

                x_all = ebuf.tile([P, nt, D], F32R, tag="x_all")
                for j in range(nt):
                    if apply_beta:
                        xtmp = ebuf.tile([P, D], F32, tag="xtmp")
                        nc.scalar.activation(
                            out=xtmp[:], in_=e_all[:, j, :], func=AF.Identity,
                            bias=nmurs[:, j:j + 1], scale=rs[:, j:j + 1])
                        nc.vector.tensor_tensor(
                            out=x_all[:, j, :], in0=xtmp[:], in1=beta_t[:],
                            op=ALU.add)
                    else:
                        nc.scalar.activation(
                            out=x_all[:, j, :], in_=e_all[:, j, :],
                            func=AF.Identity,
                            bias=nmurs[:, j:j + 1], scale=rs[:, j:j + 1])

                if debug and b == 0:
                    nc.gpsimd.dma_start(
                        out=dbg["x"][:, :],
                        in_=x_all[:].bitcast(F32).rearrange("p a d -> p (a d)"))

                if stages_eff < 3:
                    return
                # --- x^T via PE transpose: xT[p, c, j*128+q] = x[q, j, c*128+p]
                xt = ebuf.tile([P, dk, s_len], F32R, tag="xt")
                for j in range(nt):
                    pst = ps_misc.tile([P, dk, P], F32R, tag="ps_misc")
                    for c in range(dk):
                        nc.tensor.transpose(
                            out=pst[:, c, :],
                            in_=x_all[:, j, c * P:(c + 1) * P],
                            identity=ident[:])
                    nc.vector.tensor_copy(
                        out=xt[:, :, j * P:(j + 1) * P], in_=pst[:])

                if debug and b == 0:
                    nc.gpsimd.dma_start(
                        out=dbg["xt"][:, :],
                        in_=xt[:].bitcast(F32).rearrange("p a d -> p (a d)"))

                if stages_eff < 4:
                    return
                # --- projections ---
                qt = proj.tile([P, dk, s_len], F32R, tag="qt")
                kt = proj.tile([P, dk, s_len], F32R, tag="kt")
                for wname, dst in (("wq", qt), ("wk", kt)):
                    wt = w_r[wname]
                    for dj in range(dk):
                        for ch in range(nchunk):
                            ps = ps_misc.tile([P, ckw], F32, tag="ps_misc")
                            for di in range(dk):
                                nc.tensor.matmul(
                                    out=ps[:],
                                    lhsT=wt[:, di, dj * P:(dj + 1) * P],
                                    rhs=xt[:, di, ch * ckw:(ch + 1) * ckw],
                                    start=(di == 0), stop=(di == dk - 1))
                            nc.scalar.copy(
                                out=dst[:, dj, ch * ckw:(ch + 1) * ckw],
                                in_=ps[:])

                v16 = proj.tile([P, nt, D], F16, tag="v16")
                for j in range(nt):
                    ps = ps_misc.tile([P, D], F32, tag="ps_misc")
                    for di in range(dk):
                        nc.tensor.matmul(
                            out=ps[:],
                            lhsT=xt[:, di, j * P:(j + 1) * P],
                            rhs=w_r["wv"][:, di, :],
                            start=(di == 0), stop=(di == dk - 1))
                    nc.scalar.copy(out=v16[:, j, :], in_=ps[:])

                if debug and b == 0:
                    nc.gpsimd.dma_start(
                        out=dbg["qt"][:, :],
                        in_=qt[:].bitcast(F32).rearrange("p a d -> p (a d)"))
                    nc.gpsimd.dma_start(
                        out=dbg["kt"][:, :],
                        in_=kt[:].bitcast(F32).rearrange("p a d -> p (a d)"))
                    vf = ebuf.tile([P, nt, D], F32, tag="dbg_vf")
                    nc.vector.tensor_copy(out=vf[:], in_=v16[:])
                    nc.gpsimd.dma_start(
                        out=dbg["v"][:, :],
                        in_=vf[:].rearrange("p a d -> p (a d)"))

                if stages_eff < 5:
                    return
                # --- attention, software-pipelined over q-tiles ---
                # stage A(j): S matmuls; stage B(j): softmax+transpose;
                # stage C(j): PV matmuls + output. Emission order staggers
                # A(j+1) ahead of C(j) so the in-order PE never stalls on
                # the DMA-transposed P^T.
                nm = ebuf.tile([P, nt], F32, tag="nm")
                ls = ebuf.tile([P, nt], F32, tag="ls")
                rr = ebuf.tile([P, nt], F32, tag="rr")

                s_tiles = [None] * nt
                pt_tiles = [None] * nt

                def attn_a(j):
                    s_ps = ps_s.tile([P, nchunk, 512], F32, tag="s_ps")
                    s_tiles[j] = s_ps
                    for ch in range(nchunk):
                        for h in range(dk):
                            nc.tensor.matmul(
                                out=s_ps[:, ch, :ckw],
                                lhsT=qt[:, h, j * P:(j + 1) * P],
                                rhs=kt[:, h, ch * ckw:(ch + 1) * ckw],
                                start=(h == 0), stop=(h == dk - 1))

                def attn_b(j):
                    s_ps = s_tiles[j]
                    if debug and b == 0 and j == 0:
                        sf = ebuf.tile([P, s_len], F32, tag="dbg_sf")
                        nc.vector.tensor_copy(
                            out=sf[:].rearrange("p (a d) -> p a d", a=nchunk),
                            in_=s_ps[:, :, :ckw])
                        nc.gpsimd.dma_start(out=dbg["s"][:, :], in_=sf[:])
                    nc.vector.tensor_reduce(
                        out=nm[:, j:j + 1], in_=s_ps[:, :, :ckw], axis=AX.XY,
                        op=ALU.max, negate=True)
                    p16 = sm.tile([P, s_len], F16, tag="p16")
                    nc.scalar.activation(
                        out=p16[:].rearrange("p (a d) -> p a d", a=nchunk),
                        in_=s_ps[:, :, :ckw], func=AF.Exp,
                        bias=nm[:, j:j + 1], scale=1.0,
                        accum_out=ls[:, j:j + 1])
                    nc.vector.reciprocal(out=rr[:, j:j + 1], in_=ls[:, j:j + 1])
                    pt16 = sm.tile([P, nt, P], F16, tag="pt16")
                    pt_tiles[j] = pt16
                    nc.sync.dma_start_transpose(pt16[:], p16[:])
                    if debug and b == 0 and j == 0:
                        pf = ebuf.tile([P, s_len], F32, tag="dbg_pf")
                        nc.vector.tensor_copy(out=pf[:], in_=p16[:])
                        nc.gpsimd.dma_start(out=dbg["p"][:, :], in_=pf[:])
                        ptf = ebuf.tile([P, nt * P], F32, tag="dbg_ptf")
                        nc.vector.tensor_copy(
                            out=ptf[:], in_=pt16[:].rearrange("p a d -> p (a d)"))
                        nc.gpsimd.dma_start(out=dbg["pt"][:, :], in_=ptf[:])

                def attn_c(j):
                    if stages_eff < 6:
                        return
                    pt16 = pt_tiles[j]
                    o_ps = ps_misc.tile([P, D], F32, tag="ps_misc")
                    for k in range(nt):
                        nc.tensor.matmul(
                            out=o_ps[:],
                            lhsT=pt16[:, k, :],
                            rhs=v16[:, k, :],
                            start=(k == 0), stop=(k == nt - 1))
                    o_sb = sm.tile([P, D], F32, tag="o_sb")
                    nc.vector.tensor_scalar_mul(
                        out=o_sb[:], in0=o_ps[:],
                        scalar1=rr[:, j:j + 1])
                    row = (b * nt + j) * P
                    nc.sync.dma_start(
                        out=out_d[row:row + P, :], in_=o_sb[:])

                attn_a(0)
                attn_b(0)
                for j in range(1, nt):
                    attn_a(j)
                    attn_b(j)
                    attn_c(j - 1)
                attn_c(nt - 1)


            if stages < 0:
                with tc.For_i(0, -stages, 1):
                    emit_batch(0)
            else:
                for b in range(b_per_core):
                    emit_batch(b)

    fix_fat_waits(nc)
    return nc


_CACHE = {}


def _get_module(b_per_core, s_len, voc, apply_beta, stages=99):
    key = (b_per_core, s_len, voc, apply_beta, stages)
    if key not in _CACHE:
        _CACHE[key] = build(b_per_core, s_len, voc, apply_beta, stages=stages)
    return _CACHE[key]


def kernel(input, word_emb, pos_emb, gamma, beta, Wk, Wq, Wv):
    input = np.asarray(input)
    word_emb = np.ascontiguousarray(np.asarray(word_emb, dtype=np.float32))
    pos_emb = np.asarray(pos_emb, dtype=np.float32)
    gamma = np.asarray(gamma, dtype=np.float32)
    beta = np.asarray(beta, dtype=np.float32)
    Wk = np.asarray(Wk, dtype=np.float32)
    Wq = np.asarray(Wq, dtype=np.float32)
    Wv = np.asarray(Wv, dtype=np.float32)

    B, S = input.shape
    voc, d = word_emb.shape
    assert d == D
    b_per_core = B // N_CORES
    nt = S // P

    # fold gamma (scales x along d) and 1/sqrt(D) into the projection weights
    g64 = gamma.astype(np.float64)
    wq_s = (Wq.astype(np.float64) * g64[:, None] / math.sqrt(D)).astype(np.float32)
    wk_s = (Wk.astype(np.float64) * g64[:, None]).astype(np.float32)
    wv_s = (Wv.astype(np.float64) * g64[:, None]).astype(np.float32)

    apply_beta = bool(np.any(beta != 0.0))
    pos_c = np.ascontiguousarray(pos_emb[:S])

    nc = _get_module(b_per_core, S, voc, apply_beta)

    ids32 = input.astype(np.int32)  # [B, S]
    in_maps = []
    for c in range(N_CORES):
        shard = ids32[c * b_per_core:(c + 1) * b_per_core]       # [bpc, S]
        ids_col = np.ascontiguousarray(
            shard.reshape(b_per_core * nt, P).T)                 # [128, bpc*nt]
        m = {
            "ids": ids_col,
            "wemb": word_emb,
            "pos": pos_c,
            "wq": wq_s,
            "wk": wk_s,
            "wv": wv_s,
        }
        if apply_beta:
            # gamma is folded into the projection weights, so the device
            # kernel computes (xhat + b) @ (gamma*W). Feeding b = beta/gamma
            # makes that equal xhat@(gamma*W) + beta@W, the reference value.
            beta_eff = (beta.astype(np.float64)
                        / np.where(g64 == 0.0, 1.0, g64)).astype(np.float32)
            m["beta_b"] = np.ascontiguousarray(
                np.broadcast_to(beta_eff, (P, D)).astype(np.float32))
        in_maps.append(m)

    res = run_bass_kernel_spmd(nc, in_maps, core_ids=list(range(N_CORES)))
    out = np.concatenate(
        [r["out"].reshape(b_per_core, S, D) for r in res.results], axis=0)
    return out



# revision 64
# speedup vs baseline: 1.3565x; 1.3565x over previous
"""MiniBert (embed + LayerNorm + single-head attention) on 8 TRN2 NeuronCores.

Strategy: data-parallel over batch (4 sequences per core); embedding table,
pos table and projection matrices replicated to every core.

Key algebraic restructuring vs the straightforward version:
  S = x Wq Wk^T x^T / sqrt(D) = x A x^T with A folded on the host
  (A = diag(g) Wq Wk^T diag(g) / sqrt(D)). Since LayerNorm output rows have
  zero mean, the rank-1 row/col means of A contribute only softmax-invariant
  row constants, so A is double-centered on the host; that both removes one
  of the two Q/K projections on device and makes A fp16-friendly (entries
  collapse from ~5.7 +- 0.4 to ~0 +- 0.4).

Per-core pipeline (per sequence, all fp16 operands, fp32 accumulation):
  gather word_emb rows (indirect DMA, fp16) -> += pos_emb (DVE)
  LayerNorm per token tile: bn_stats/bn_aggr (DVE), rsqrt via exp(-.5 ln) on
  ACT, normalize on ACT -> x fp16; x^T per token tile via DMA transpose
  G^T = A^T-slices @ x^T; V = x^T-slices @ Wv'  (PE, fp16)
  S(j) = G^T.T @ x^T in PSUM fp32; row-max (DVE), P = exp(S-max) fp16 (ACT)
  with accumulated row sums; P^T via DMA transpose; O = P^T.T @ V in PSUM,
  scaled by 1/rowsum (ACT) -> fp16 out.
Sequences are software-pipelined: the embed/LN/transpose chain of sequence
b+1 is emitted inside the attention loop of sequence b so the PE never waits
on the front-end; a few warm-up matmuls ahead of the first gather ramp the
PE clock during the initial DMA wait.
"""
import math
import numpy as np

from concourse import bass, mybir
import concourse.tile as tile
from concourse.bass_utils import run_bass_kernel_spmd
from concourse.masks import make_identity

P = 128
D = 512
VOC = 32000
N_CORES = 8

F32 = mybir.dt.float32
F16 = mybir.dt.float16
I32 = mybir.dt.int32

AF = mybir.ActivationFunctionType
ALU = mybir.AluOpType
AX = mybir.AxisListType


def fix_fat_waits(nc, max_waits=1):
    """Walrus rejects instructions carrying more than ~1 semaphore wait. Tile
    occasionally emits joins (notably the kernel-tail drain) with one wait per
    producing processor. Split the extras into a chain of single-wait NoOps on
    the same engine, inserted immediately before the original instruction."""
    n_new = 0
    for bb in nc.main_func.blocks:
        insts = bb.instructions
        i = 0
        while i < len(insts):
            ins = insts[i]
            si = ins.sync_info
            if si and si.on_wait and len(si.on_wait) > max_waits:
                waits = list(si.on_wait)
                keep = waits[-max_waits:]
                extra = waits[:-max_waits]
                ins.sync_info = mybir.SyncInfo(
                    on_wait=keep, on_update=list(si.on_update or []))
                for j, w in enumerate(extra):
                    nop = mybir.InstNoOp(name=f"W-split-{n_new}", ins=[], outs=[])
                    n_new += 1
                    nop.engine = ins.engine
                    nop.sync_info = mybir.SyncInfo(on_wait=[w], on_update=[])
                    insts.insert(i + j, nop)
                i += len(extra)
            i += 1
    return n_new


def build(b_per_core: int, s_len: int, voc: int, apply_beta: bool,
          debug: bool = False, stages: int = 99, warm_mms: int = 8):
    """Build the per-core SPMD program. All cores run this same module."""
    nt = s_len // P          # token tiles per sequence
    dk = D // P              # feature tiles (4)
    nch = 2                  # free-dim chunks for the N=s_len matmuls
    ckw = s_len // nch

    nc = bass.Bass(dynamic_dma_scratch_size=32768)

    dbg = {}
    if debug:
        dbg["x"] = nc.dram_tensor("dbg_x", [P, nt * D], F32, kind="ExternalOutput")
        dbg["xt"] = nc.dram_tensor("dbg_xt", [P, dk * s_len], F32, kind="ExternalOutput")
        dbg["gt"] = nc.dram_tensor("dbg_gt", [P, dk * s_len], F32, kind="ExternalOutput")
        dbg["v"] = nc.dram_tensor("dbg_v", [P, nt * D], F32, kind="ExternalOutput")
        dbg["s"] = nc.dram_tensor("dbg_s", [P, s_len], F32, kind="ExternalOutput")
        dbg["p"] = nc.dram_tensor("dbg_p", [P, s_len], F32, kind="ExternalOutput")

    ids_d = nc.dram_tensor("ids", [P, b_per_core * nt], I32, kind="ExternalInput")
    wemb_d = nc.dram_tensor("wemb", [voc, D], F16, kind="ExternalInput")
    pos_d = nc.dram_tensor("pos", [s_len, D], F16, kind="ExternalInput")
    a_d = nc.dram_tensor("amat", [D, D], F16, kind="ExternalInput")
    wv_d = nc.dram_tensor("wv", [D, D], F16, kind="ExternalInput")
    if apply_beta:
        gb_d = nc.dram_tensor("gbias", [P, dk], F32, kind="ExternalInput")
        vb_d = nc.dram_tensor("vbias", [P, D], F16, kind="ExternalInput")
    out_d = nc.dram_tensor("out", [b_per_core * s_len, D], F16, kind="ExternalOutput")

    with tile.TileContext(nc) as tc:
        with (
            tc.tile_pool(name="pers", bufs=1) as pers,
            tc.tile_pool(name="ebuf", bufs=2) as ebuf,
            tc.tile_pool(name="tbuf", bufs=2) as tbuf,
            tc.tile_pool(name="sm", bufs=4) as sm,
            tc.tile_pool(name="ps_s", bufs=2, space="PSUM") as ps_s,
            tc.tile_pool(name="ps_gv", bufs=2, space="PSUM") as ps_gv,
            tc.tile_pool(name="ps_o", bufs=2, space="PSUM") as ps_o,
        ):
            # ---------------- preamble: persistent tiles ----------------
            # The simulated DMA engine pool drains transfers roughly in
            # dispatch order, so keep the early-needed pieces small and first:
            # ids, then per-token-tile pos slices (sync queue) interleaving
            # with the first gathers; wv/a stream on the scalar queue.
            ids_t = pers.tile([P, b_per_core * nt], I32, tag="ids")
            nc.sync.dma_start(out=ids_t[:], in_=ids_d[:, :])

            pos_t = []
            for j in range(nt):
                pt_ = pers.tile([P, D], F16, tag=f"pos{j}", name=f"pos{j}_t")
                pos_t.append(pt_)
            wv_t = pers.tile([P, dk, D], F16, tag="wv")
            a_t = pers.tile([P, dk, D], F16, tag="a")
            for j in range(nt):
                nc.sync.dma_start(out=pos_t[j][:],
                                  in_=pos_d[j * P:(j + 1) * P, :])
            nc.scalar.dma_start(
                out=wv_t[:], in_=wv_d.rearrange("(a p) n -> p a n", p=P))
            nc.scalar.dma_start(
                out=a_t[:], in_=a_d.rearrange("(a p) n -> p a n", p=P))

            if apply_beta:
                gb_t = pers.tile([P, dk], F32, tag="gb")
                nc.sync.dma_start(out=gb_t[:], in_=gb_d[:, :])
                vb_t = pers.tile([P, D], F16, tag="vb")
                nc.sync.dma_start(out=vb_t[:], in_=vb_d[:, :])

            epsb = pers.tile([P, 1], F32, tag="epsb")
            nc.vector.memset(epsb[:], 1e-5)
            ident_f = pers.tile([P, P], F32, tag="ident_f")
            make_identity(nc, ident_f[:])
            ident = pers.tile([P, P], F16, tag="ident")
            nc.vector.tensor_copy(out=ident[:], in_=ident_f[:])
            # touch the ACT function table early so the (slow) table load
            # overlaps the preamble DMAs instead of the first LayerNorm.
            actw = pers.tile([P, 1], F32, tag="actw")
            nc.scalar.activation(out=actw[:], in_=epsb[:], func=AF.Exp,
                                 bias=0.0, scale=1.0)

            # warm-up matmuls: ramp the PE clock while the first sequence's
            # embedding gathers are in flight. Results are never read and the
            # operands' values don't matter; epsb is memset first on the DVE
            # so the wait chain is minimal.
            if warm_mms > 0 and stages >= 3:
                wscr = pers.tile([P, ckw], F16, tag="wscr")
                nc.vector.memset(wscr[:], 0.0)
                for w in range(warm_mms):
                    wps = ps_gv.tile([P, ckw], F32, tag="gv")
                    nc.tensor.matmul(out=wps[:], lhsT=wscr[:, 0:P],
                                     rhs=wscr[:], start=True, stop=True)

            # ---------------- per-sequence emitters ----------------
            seq = {}   # per-seq tiles, rotated via pools

            tpc = nt // nch   # token tiles per chunk

            def embed_tile(b, j):
                """Gather+LN+transpose chain for token tile j of sequence b."""
                if stages < 1:
                    return
                if j == 0:
                    shapes = {"v": (tbuf, [P, nt, D], F16)}
                    for jj in range(nt):
                        shapes[f"e{jj}"] = (ebuf, [P, D], F16)
                        shapes[f"x{jj}"] = (ebuf, [P, D], F16)
                        shapes[f"s6_{jj}"] = (ebuf, [P, 6], F32)
                        shapes[f"mv{jj}"] = (ebuf, [P, 2], F32)
                        shapes[f"lnv{jj}"] = (ebuf, [P, 1], F32)
                        shapes[f"rs{jj}"] = (ebuf, [P, 1], F32)
                        shapes[f"nmu{jj}"] = (ebuf, [P, 1], F32)
                        shapes[f"nm{jj}"] = (ebuf, [P, 1], F32)
                        shapes[f"ls{jj}"] = (ebuf, [P, 1], F32)
                        shapes[f"rr{jj}"] = (ebuf, [P, 1], F32)
                    for ch in range(nch):
                        shapes[f"xt{ch}"] = (tbuf, [P, dk, ckw], F16)
                        shapes[f"gt{ch}"] = (tbuf, [P, dk, ckw], F16)
                    seq[b] = {k: pool.tile(shp, dt, tag=k, name=f"{k}_t")
                              for k, (pool, shp, dt) in shapes.items()}
                t = seq[b]
                e, x = t[f"e{j}"], t[f"x{j}"]
                nc.gpsimd.indirect_dma_start(
                    out=e[:], out_offset=None, in_=wemb_d[:],
                    in_offset=bass.IndirectOffsetOnAxis(
                        ap=ids_t[:, b * nt + j: b * nt + j + 1], axis=0))
                if stages < 2:
                    return
                mv, rs, nmu = t[f"mv{j}"], t[f"rs{j}"], t[f"nmu{j}"]
                nc.vector.tensor_tensor(
                    out=e[:], in0=e[:], in1=pos_t[j][:], op=ALU.add)
                nc.vector.bn_stats(out=t[f"s6_{j}"][:], in_=e[:])
                nc.vector.bn_aggr(out=mv[:], in_=t[f"s6_{j}"][:])
                # rs = exp(-0.5*ln(var+eps)) == rsqrt(var+eps)
                nc.scalar.activation(
                    out=t[f"lnv{j}"][:], in_=mv[:, 1:2],
                    func=AF.Ln, bias=epsb[:, 0:1], scale=1.0)
                nc.scalar.activation(
                    out=rs[:], in_=t[f"lnv{j}"][:],
                    func=AF.Exp, bias=0.0, scale=-0.5)
                nc.vector.tensor_tensor(
                    out=nmu[:], in0=mv[:, 0:1], in1=rs[:], op=ALU.mult)
                nc.vector.tensor_scalar_mul(
                    out=nmu[:], in0=nmu[:], scalar1=-1.0)
                nc.scalar.activation(
                    out=x[:], in_=e[:], func=AF.Identity,
                    bias=nmu[:], scale=rs[:])

            def embed_xpose(b, j):
                """x^T for tile j on the PE (fp16 transpose, 53ns per 128x128
                tile) + DVE copy out of PSUM. Emitted one pipeline step after
                embed_tile(b, j) so the PE never waits on the LN chain."""
                if stages < 2:
                    return
                t = seq[b]
                x = t[f"x{j}"]
                pst = ps_gv.tile([P, dk, P], F16, tag="gv", name="ps_t")
                for c in range(dk):
                    nc.tensor.transpose(
                        out=pst[:, c, :], in_=x[:, c * P:(c + 1) * P],
                        identity=ident[:])
                nc.vector.tensor_copy(
                    out=t[f"xt{j // tpc}"][:, :, (j % tpc) * P:(j % tpc + 1) * P],
                    in_=pst[:])

            def vproj(b, j, pool=None):
                if stages < 3:
                    return
                t = seq[b]
                xt = t[f"xt{j // tpc}"]
                jo = (j % tpc) * P
                ps = (ps_o.tile([P, D], F32, tag="o", name="ps_v")
                      if pool is ps_o else
                      ps_gv.tile([P, D], F32, tag="gv", name="ps_v"))
                for di in range(dk):
                    nc.tensor.matmul(
                        out=ps[:], lhsT=xt[:, di, jo:jo + P],
                        rhs=wv_t[:, di, :], start=(di == 0), stop=(di == dk - 1))
                if apply_beta:
                    nc.vector.tensor_tensor(
                        out=t["v"][:, j, :], in0=ps[:], in1=vb_t[:],
                        op=ALU.add)
                elif j % 2 == 0:
                    nc.vector.tensor_copy(out=t["v"][:, j, :], in_=ps[:])
                else:
                    nc.scalar.copy(out=t["v"][:, j, :], in_=ps[:])

            def gproj(b, ch, djs=None):
                if stages < 3:
                    return
                t = seq[b]
                xt = t[f"xt{ch}"]
                for dj in (range(dk) if djs is None else djs):
                    ps = ps_gv.tile([P, ckw], F32, tag="gv", name="ps_g")
                    for di in range(dk):
                        nc.tensor.matmul(
                            out=ps[:], lhsT=a_t[:, di, dj * P:(dj + 1) * P],
                            rhs=xt[:, di, :],
                            start=(di == 0), stop=(di == dk - 1))
                    if apply_beta:
                        nc.scalar.activation(
                            out=t[f"gt{ch}"][:, dj, :], in_=ps[:],
                            func=AF.Identity,
                            bias=gb_t[:, dj:dj + 1], scale=1.0)
                    elif dj % 2 == 0:
                        nc.scalar.copy(out=t[f"gt{ch}"][:, dj, :], in_=ps[:])
                    else:
                        nc.vector.tensor_copy(
                            out=t[f"gt{ch}"][:, dj, :], in_=ps[:])

            def emit_dbg(b):
                if debug and b == 0 and stages >= 3:
                    t = seq[b]
                    fv = ebuf.tile([P, nt * D], F32, tag="dbg_v")
                    nc.vector.tensor_copy(
                        out=fv[:], in_=t["v"][:].rearrange("p a d -> p (a d)"))
                    nc.gpsimd.dma_start(out=dbg["v"][:, :], in_=fv[:])
                    fx = ebuf.tile([P, nt * D], F32, tag="dbg_x")
                    for jj in range(nt):
                        nc.vector.tensor_copy(
                            out=fx[:, jj * D:(jj + 1) * D], in_=t[f"x{jj}"][:])
                    nc.gpsimd.dma_start(out=dbg["x"][:, :], in_=fx[:])
                    for name in ("xt", "gt"):
                        f = ebuf.tile([P, dk, s_len], F32, tag=f"dbg_{name}")
                        for ch in range(nch):
                            nc.vector.tensor_copy(
                                out=f[:, :, ch * ckw:(ch + 1) * ckw],
                                in_=t[f"{name}{ch}"][:])
                        nc.gpsimd.dma_start(
                            out=dbg[name][:, :],
                            in_=f[:].rearrange("p a d -> p (a d)"))

            s_tiles = [None] * nt
            pt_tiles = [None] * nt

            def attn_a(b, j):
                if stages < 4:
                    return
                t = seq[b]
                gt = t[f"gt{j // tpc}"]
                jo = (j % tpc) * P
                s_ps = ps_s.tile([P, nch, ckw], F32, tag="s", name="ps_s_t")
                s_tiles[j] = s_ps
                for ch in range(nch):
                    for di in range(dk):
                        nc.tensor.matmul(
                            out=s_ps[:, ch, :],
                            lhsT=gt[:, di, jo:jo + P],
                            rhs=t[f"xt{ch}"][:, di, :],
                            start=(di == 0), stop=(di == dk - 1))

            def attn_b(b, j):
                if stages < 5:
                    return
                t = seq[b]
                s_ps = s_tiles[j]
                if debug and b == 0 and j == 0:
                    sf = ebuf.tile([P, s_len], F32, tag="dbg_s")
                    nc.vector.tensor_copy(
                        out=sf[:].rearrange("p (a d) -> p a d", a=nch),
                        in_=s_ps[:])
                    nc.gpsimd.dma_start(out=dbg["s"][:, :], in_=sf[:])
                nc.vector.tensor_reduce(
                    out=t[f"nm{j}"][:], in_=s_ps[:], axis=AX.XY,
                    op=ALU.max, negate=True)
                p16 = sm.tile([P, s_len], F16, tag="p16")
                nc.scalar.activation(
                    out=p16[:].rearrange("p (a d) -> p a d", a=nch),
                    in_=s_ps[:], func=AF.Exp,
                    bias=t[f"nm{j}"][:], scale=1.0,
                    accum_out=t[f"ls{j}"][:])
                nc.vector.reciprocal(
                    out=t[f"rr{j}"][:], in_=t[f"ls{j}"][:])
                pt16 = sm.tile([P, nt, P], F16, tag="pt16")
                pt_tiles[j] = pt16
                nc.sync.dma_start_transpose(pt16[:], p16[:])
                if debug and b == 0 and j == 0:
                    pf = ebuf.tile([P, s_len], F32, tag="dbg_p")
                    nc.vector.tensor_copy(out=pf[:], in_=p16[:])
                    nc.gpsimd.dma_start(out=dbg["p"][:, :], in_=pf[:])

            def attn_c(b, j):
                if stages < 6:
                    return
                t = seq[b]
                pt16 = pt_tiles[j]
                o_ps = ps_o.tile([P, D], F32, tag="o")
                for k in range(nt):
                    nc.tensor.matmul(
                        out=o_ps[:], lhsT=pt16[:, k, :], rhs=t["v"][:, k, :],
                        start=(k == 0), stop=(k == nt - 1))
                o16 = sm.tile([P, D], F16, tag="o16")
                nc.scalar.activation(
                    out=o16[:], in_=o_ps[:], func=AF.Identity,
                    bias=0.0, scale=t[f"rr{j}"][:])
                row = (b * nt + j) * P
                nc.sync.dma_start(out=out_d[row:row + P, :], in_=o16[:])

            def emit_attn(b):
                """Attention for seq b, with the embed chain of seq b+1
                emitted inside the loop and PV lagging S by 2 q-tiles; the
                last two PV stages interleave with seq b+1's projections so
                the PE never waits on the softmax->transpose chain."""
                nxt = b + 1 if b + 1 < b_per_core else None
                lag = min(2, nt - 1)
                spread = nxt is not None and nt == 8
                for j in range(nt):
                    if j >= lag:
                        attn_c(b, j - lag)
                    if nxt is not None:
                        embed_tile(nxt, j)
                        if j >= 1:
                            embed_xpose(nxt, j - 1)
                        if spread and j >= 2:
                            vproj(nxt, j - 2)
                    attn_a(b, j)
                    attn_b(b, j)
                    if spread and j >= 6:
                        gproj(nxt, 0, djs=[2 * (j - 6), 2 * (j - 6) + 1])
                tail = list(range(nt - lag, nt))
                if spread:
                    attn_c(b, tail[0])
                    embed_xpose(nxt, nt - 1)
                    vproj(nxt, nt - 2)
                    attn_c(b, tail[1])
                    vproj(nxt, nt - 1)
                    gproj(nxt, 1)
                elif nxt is not None:
                    attn_c(b, tail[0])
                    embed_xpose(nxt, nt - 1)
                    for j in range(min(4, nt)):
                        vproj(nxt, j)
                    for j in tail[1:-1]:
                        attn_c(b, j)
                    gproj(nxt, 0)
                    attn_c(b, tail[-1])
                    for j in range(4, nt):
                        vproj(nxt, j)
                    gproj(nxt, 1)
                else:
                    for j in tail:
                        attn_c(b, j)
                del s_tiles[:]
                s_tiles.extend([None] * nt)
                if b in seq and nxt is not None:
                    del seq[b]

            # ---------------- schedule ----------------
            if b_per_core > 0 and stages >= 1:
                if nt == 8:
                    # prologue: per-tile stream — V(j) needs only tile j's
                    # transpose, so the PE gets dense work as soon as the
                    # first LN lands. V uses the (otherwise idle) ps_o pool
                    # so V and G psum recycling don't serialize.
                    embed_tile(0, 0)
                    embed_tile(0, 1)
                    embed_xpose(0, 0)
                    vproj(0, 0, pool=ps_o)
                    embed_tile(0, 2)
                    embed_xpose(0, 1)
                    vproj(0, 1, pool=ps_o)
                    embed_tile(0, 3)
                    embed_xpose(0, 2)
                    vproj(0, 2, pool=ps_o)
                    embed_tile(0, 4)
                    embed_xpose(0, 3)
                    vproj(0, 3, pool=ps_o)
                    embed_tile(0, 5)
                    embed_xpose(0, 4)
                    vproj(0, 4, pool=ps_o)
                    gproj(0, 0, djs=[0, 1])
                    embed_tile(0, 6)
                    embed_xpose(0, 5)
                    vproj(0, 5, pool=ps_o)
                    gproj(0, 0, djs=[2, 3])
                    embed_tile(0, 7)
                    embed_xpose(0, 6)
                    vproj(0, 6, pool=ps_o)
                    embed_xpose(0, 7)
                    vproj(0, 7, pool=ps_o)
                    gproj(0, 1)
                else:
                    for j in range(nt):
                        embed_tile(0, j)
                    for j in range(nt):
                        embed_xpose(0, j)
                    for j in range(min(4, nt)):
                        vproj(0, j)
                    gproj(0, 0)
                    for j in range(4, nt):
                        vproj(0, j)
                    gproj(0, 1)
                emit_dbg(0)
                for b in range(b_per_core):
                    emit_attn(b)

    fix_fat_waits(nc)
    return nc


_CACHE = {}


def _get_module(b_per_core, s_len, voc, apply_beta, stages=99):
    key = (b_per_core, s_len, voc, apply_beta, stages)
    if key not in _CACHE:
        _CACHE[key] = build(b_per_core, s_len, voc, apply_beta, stages=stages)
    return _CACHE[key]


def _host_fold(word_emb, pos_emb, gamma, beta, Wk, Wq, Wv, S):
    """Host-side weight folding: A = diag(g) Wq Wk^T diag(g)/sqrt(D), double
    centered (exact under zero-row-mean LN output); beta handled via rank-1
    bias terms on G and V."""
    g64 = gamma.astype(np.float64)
    b64 = beta.astype(np.float64)
    M = Wq.astype(np.float64) @ Wk.astype(np.float64).T / math.sqrt(D)
    A = g64[:, None] * M * g64[None, :]
    A = A - A.mean(0, keepdims=True)
    A = A - A.mean(1, keepdims=True)
    a16 = np.ascontiguousarray(A.astype(np.float16))

    wv16 = np.ascontiguousarray((g64[:, None] * Wv.astype(np.float64))
                                .astype(np.float16))

    apply_beta = bool(np.any(beta != 0.0))
    folds = {"amat": a16, "wv": wv16}
    if apply_beta:
        v = (b64 @ M) * g64
        v = v - v.mean()
        folds["gbias"] = np.ascontiguousarray(
            v.reshape(D // P, P).T.astype(np.float32))
        vb = (b64 @ Wv.astype(np.float64)).astype(np.float16)
        folds["vbias"] = np.ascontiguousarray(
            np.broadcast_to(vb, (P, D)).astype(np.float16))
    return folds, apply_beta


def kernel(input, word_emb, pos_emb, gamma, beta, Wk, Wq, Wv):
    input = np.asarray(input)
    word_emb = np.asarray(word_emb, dtype=np.float32)
    pos_emb = np.asarray(pos_emb, dtype=np.float32)
    gamma = np.asarray(gamma, dtype=np.float32)
    beta = np.asarray(beta, dtype=np.float32)
    Wk = np.asarray(Wk, dtype=np.float32)
    Wq = np.asarray(Wq, dtype=np.float32)
    Wv = np.asarray(Wv, dtype=np.float32)

    B, S = input.shape
    voc, d = word_emb.shape
    assert d == D
    b_per_core = B // N_CORES
    nt = S // P

    folds, apply_beta = _host_fold(word_emb, pos_emb, gamma, beta, Wk, Wq, Wv, S)
    wemb16 = np.ascontiguousarray(word_emb.astype(np.float16))
    pos16 = np.ascontiguousarray(pos_emb[:S].astype(np.float16))

    nc = _get_module(b_per_core, S, voc, apply_beta)

    ids32 = input.astype(np.int32)  # [B, S]
    in_maps = []
    for c in range(N_CORES):
        shard = ids32[c * b_per_core:(c + 1) * b_per_core]       # [bpc, S]
        ids_col = np.ascontiguousarray(
            shard.reshape(b_per_core * nt, P).T)                 # [128, bpc*nt]
        m = {"ids": ids_col, "wemb": wemb16, "pos": pos16, **folds}
        in_maps.append(m)

    res = run_bass_kernel_spmd(nc, in_maps, core_ids=list(range(N_CORES)))
    out = np.concatenate(
        [r["out"].reshape(b_per_core, S, D) for r in res.results],
        axis=0).astype(np.float32)
    return out


# revision 102
# speedup vs baseline: 1.5454x; 1.1392x over previous
"""MiniBert (embed + LayerNorm + single-head attention) on 8 TRN2 NeuronCores.

Strategy: data-parallel over batch (4 sequences per core); embedding table,
pos table and projection matrices replicated to every core.

Key algebraic restructuring vs the straightforward version:
  S = x Wq Wk^T x^T / sqrt(D) = x A x^T with A folded on the host
  (A = diag(g) Wq Wk^T diag(g) / sqrt(D)). Since LayerNorm output rows have
  zero mean, the rank-1 row/col means of A contribute only softmax-invariant
  row constants, so A is double-centered on the host; that both removes one
  of the two Q/K projections on device and makes A fp16-friendly (entries
  collapse from ~5.7 +- 0.4 to ~0 +- 0.4).

Per-core pipeline (per sequence, all fp16 operands, fp32 accumulation):
  gather word_emb rows (indirect DMA, fp16) -> += pos_emb (DVE)
  LayerNorm per token tile: bn_stats/bn_aggr (DVE), rsqrt via exp(-.5 ln) on
  ACT, normalize on ACT -> x fp16; x^T per token tile via PE fp16 transposes
  (53ns per 128x128) + DVE copy out of PSUM
  G^T = A^T-slices @ x^T; V = x^T-slices @ Wv'  (PE, fp16)
  S(j) = G^T.T @ x^T in PSUM fp32; row-max (DVE), P = exp(S-max) fp16 (ACT)
  with accumulated row sums; P^T via DMA transpose; O = P^T.T @ V in PSUM,
  scaled by 1/rowsum (ACT) -> fp16 out.
Scheduling: the whole kernel is software-pipelined for the in-order engine
queues. The embed/LN/transpose chain and the G/V projections of sequence b+1
are spread through the attention loop of sequence b; PV(j) trails S(j) by 7
q-tiles (all eight P^T tiles triple-buffered in SBUF) so the PE never waits
on the softmax -> DMA-transpose chain; PSUM pools are split S/GV/O so their
buffer recycling chains don't serialize; warm-up matmuls ahead of the first
gather ramp the PE clock during the initial DMA wait.
"""
import math
import numpy as np

from concourse import bass, mybir
import concourse.tile as tile
from concourse.bass_utils import run_bass_kernel_spmd
from concourse.masks import make_identity

P = 128
XT_DMA = False
D = 512
VOC = 32000
N_CORES = 8

F32 = mybir.dt.float32
F16 = mybir.dt.float16
I32 = mybir.dt.int32

AF = mybir.ActivationFunctionType
ALU = mybir.AluOpType
AX = mybir.AxisListType


def fix_fat_waits(nc, max_waits=1):
    """Walrus rejects instructions carrying more than ~1 semaphore wait. Tile
    occasionally emits joins (notably the kernel-tail drain) with one wait per
    producing processor. Split the extras into a chain of single-wait NoOps on
    the same engine, inserted immediately before the original instruction."""
    n_new = 0
    for bb in nc.main_func.blocks:
        insts = bb.instructions
        i = 0
        while i < len(insts):
            ins = insts[i]
            si = ins.sync_info
            if si and si.on_wait and len(si.on_wait) > max_waits:
                waits = list(si.on_wait)
                keep = waits[-max_waits:]
                extra = waits[:-max_waits]
                ins.sync_info = mybir.SyncInfo(
                    on_wait=keep, on_update=list(si.on_update or []))
                for j, w in enumerate(extra):
                    nop = mybir.InstNoOp(name=f"W-split-{n_new}", ins=[], outs=[])
                    n_new += 1
                    nop.engine = ins.engine
                    nop.sync_info = mybir.SyncInfo(on_wait=[w], on_update=[])
                    insts.insert(i + j, nop)
                i += len(extra)
            i += 1
    return n_new


def build(b_per_core: int, s_len: int, voc: int, apply_beta: bool,
          debug: bool = False, stages: int = 99, warm_mms: int = 8):
    """Build the per-core SPMD program. All cores run this same module."""
    nt = s_len // P          # token tiles per sequence
    dk = D // P              # feature tiles (4)
    nch = 2                  # free-dim chunks for the N=s_len matmuls
    ckw = s_len // nch

    nc = bass.Bass()

    dbg = {}
    if debug:
        dbg["x"] = nc.dram_tensor("dbg_x", [P, nt * D], F32, kind="ExternalOutput")
        dbg["xt"] = nc.dram_tensor("dbg_xt", [P, dk * s_len], F32, kind="ExternalOutput")
        dbg["gt"] = nc.dram_tensor("dbg_gt", [P, dk * s_len], F32, kind="ExternalOutput")
        dbg["v"] = nc.dram_tensor("dbg_v", [P, nt * D], F32, kind="ExternalOutput")
        dbg["s"] = nc.dram_tensor("dbg_s", [P, s_len], F32, kind="ExternalOutput")
        dbg["p"] = nc.dram_tensor("dbg_p", [P, s_len], F32, kind="ExternalOutput")

    ids_d = nc.dram_tensor("ids", [P, b_per_core * nt], I32, kind="ExternalInput")
    wemb_d = nc.dram_tensor("wemb", [voc, D], F16, kind="ExternalInput")
    pos_d = nc.dram_tensor("pos", [s_len, D], F16, kind="ExternalInput")
    a_d = nc.dram_tensor("amat", [D, D], F16, kind="ExternalInput")
    wv_d = nc.dram_tensor("wv", [D, D], F16, kind="ExternalInput")
    if apply_beta:
        gb_d = nc.dram_tensor("gbias", [P, dk], F32, kind="ExternalInput")
        vb_d = nc.dram_tensor("vbias", [P, D], F16, kind="ExternalInput")
    out_d = nc.dram_tensor("out", [b_per_core * s_len, D], F16, kind="ExternalOutput")

    with tile.TileContext(nc) as tc:
        with (
            tc.tile_pool(name="pers", bufs=1) as pers,
            tc.tile_pool(name="ebuf", bufs=3) as ebuf,
            tc.tile_pool(name="tbuf", bufs=2) as tbuf,
            tc.tile_pool(name="sm", bufs=8) as sm,
            tc.tile_pool(name="ps_s", bufs=2, space="PSUM") as ps_s,
            tc.tile_pool(name="ps_gv", bufs=2, space="PSUM") as ps_gv,
            tc.tile_pool(name="ps_o", bufs=2, space="PSUM") as ps_o,
        ):
            # ---------------- preamble: persistent tiles ----------------
            # The simulated DMA engine pool drains transfers roughly in
            # dispatch order, so keep the early-needed pieces small and first:
            # ids, then per-token-tile pos slices (sync queue) interleaving
            # with the first gathers; wv/a stream on the scalar queue.
            ids_t = pers.tile([P, b_per_core * nt], I32, tag="ids")
            nc.sync.dma_start(out=ids_t[:], in_=ids_d[:, :])

            pos_t = []
            for j in range(nt):
                pt_ = pers.tile([P, D], F16, tag=f"pos{j}", name=f"pos{j}_t")
                pos_t.append(pt_)
            wv_t = pers.tile([P, dk, D], F16, tag="wv")
            a_t = pers.tile([P, dk, D], F16, tag="a")
            npre = 6 if nt == 8 else nt
            for j in range(npre):
                nc.sync.dma_start(out=pos_t[j][:],
                                  in_=pos_d[j * P:(j + 1) * P, :])

            def late_loads():
                # deferred so the first gathers' transfers (critical path to
                # the first LayerNorm) aren't queued behind them on the DMA
                # engines.
                for j in range(npre, nt):
                    nc.sync.dma_start(out=pos_t[j][:],
                                      in_=pos_d[j * P:(j + 1) * P, :])
                nc.sync.dma_start(
                    out=wv_t[:], in_=wv_d.rearrange("(a p) n -> p a n", p=P))
                nc.sync.dma_start(
                    out=a_t[:], in_=a_d.rearrange("(a p) n -> p a n", p=P))

            if apply_beta:
                gb_t = pers.tile([P, dk], F32, tag="gb")
                nc.sync.dma_start(out=gb_t[:], in_=gb_d[:, :])
                vb_t = pers.tile([P, D], F16, tag="vb")
                nc.sync.dma_start(out=vb_t[:], in_=vb_d[:, :])

            epsb = pers.tile([P, 1], F32, tag="epsb")
            nc.vector.memset(epsb[:], 1e-5)
            # touch the ACT function table early so the (slow) table load
            # overlaps the preamble DMAs instead of the first LayerNorm.
            actw = pers.tile([P, 1], F32, tag="actw")
            nc.scalar.activation(out=actw[:], in_=epsb[:], func=AF.Exp,
                                 bias=0.0, scale=1.0)

            ident_f = pers.tile([P, P], F32, tag="ident_f")
            ident = pers.tile([P, P], F16, tag="ident")
            wscr = pers.tile([P, ckw], F16, tag="wscr")

            def emit_ident_warm():
                """PE-clock warm-up matmuls + identity for PE transposes.
                Emitted after the first gathers so make_identity's gpsimd
                work doesn't delay the first gather dispatch on the Pool
                queue. Warm-up results are never read."""
                if warm_mms > 0 and stages >= 3:
                    nc.vector.memset(wscr[:], 0.0)
                    for w in range(warm_mms):
                        wps = ps_gv.tile([P, ckw], F32, tag="gv")
                        nc.tensor.matmul(out=wps[:], lhsT=wscr[:, 0:P],
                                         rhs=wscr[:], start=True, stop=True)
                make_identity(nc, ident_f[:])
                nc.gpsimd.tensor_copy(out=ident[:], in_=ident_f[:])

            # ---------------- per-sequence emitters ----------------
            seq = {}   # per-seq tiles, rotated via pools

            tpc = nt // nch   # token tiles per chunk

            def embed_tile(b, j):
                """Gather+LN+transpose chain for token tile j of sequence b."""
                if stages < 1:
                    return
                if j == 0:
                    shapes = {"v": (tbuf, [P, nt, D], F16)}
                    for jj in range(nt):
                        shapes[f"e{jj}"] = (ebuf, [P, D], F16)
                        shapes[f"x{jj}"] = (ebuf, [P, D], F16)
                        shapes[f"s6_{jj}"] = (ebuf, [P, 6], F32)
                        shapes[f"mv{jj}"] = (ebuf, [P, 2], F32)
                        shapes[f"lnv{jj}"] = (ebuf, [P, 1], F32)
                        shapes[f"rs{jj}"] = (ebuf, [P, 1], F32)
                        shapes[f"nmu{jj}"] = (ebuf, [P, 1], F32)
                        shapes[f"m2_{jj}"] = (ebuf, [P, 1], F32)
                        shapes[f"nm{jj}"] = (ebuf, [P, 1], F32)
                        shapes[f"ls{jj}"] = (ebuf, [P, 1], F32)
                        shapes[f"rr{jj}"] = (ebuf, [P, 1], F32)
                    for ch in range(nch):
                        shapes[f"xt{ch}"] = (tbuf, [P, dk, ckw], F16)
                        shapes[f"gt{ch}"] = (tbuf, [P, dk, ckw], F16)
                    seq[b] = {k: pool.tile(shp, dt, tag=k, name=f"{k}_t")
                              for k, (pool, shp, dt) in shapes.items()}
                t = seq[b]
                e, x = t[f"e{j}"], t[f"x{j}"]
                nc.gpsimd.indirect_dma_start(
                    out=e[:], out_offset=None, in_=wemb_d[:],
                    in_offset=bass.IndirectOffsetOnAxis(
                        ap=ids_t[:, b * nt + j: b * nt + j + 1], axis=0))
                if stages < 2:
                    return
                mv, rs, nmu = t[f"mv{j}"], t[f"rs{j}"], t[f"nmu{j}"]
                nc.vector.tensor_tensor(
                    out=e[:], in0=e[:], in1=pos_t[j][:], op=ALU.add)
                nc.vector.bn_stats(out=t[f"s6_{j}"][:], in_=e[:])
                nc.vector.bn_aggr(out=mv[:], in_=t[f"s6_{j}"][:])
                # rs = exp(-0.5*ln(var+eps)) == rsqrt(var+eps)
                nc.scalar.activation(
                    out=t[f"lnv{j}"][:], in_=mv[:, 1:2],
                    func=AF.Ln, bias=epsb[:, 0:1], scale=1.0)
                nc.scalar.activation(
                    out=rs[:], in_=t[f"lnv{j}"][:],
                    func=AF.Exp, bias=0.0, scale=-0.5)
                # m2 = -mean on DVE (parallel with Ln/Exp above), then
                # nmu = m2*rs on ACT: keeps the serial chain on one engine
                m2 = t[f"m2_{j}"]
                nc.vector.tensor_scalar_mul(
                    out=m2[:], in0=mv[:, 0:1], scalar1=-1.0)
                nc.scalar.activation(
                    out=nmu[:], in_=rs[:], func=AF.Identity,
                    bias=0.0, scale=m2[:])
                nc.scalar.activation(
                    out=x[:], in_=e[:], func=AF.Identity,
                    bias=nmu[:], scale=rs[:])

            def embed_xpose(b, j):
                """x^T for tile j on the PE (fp16 transpose, 53ns per 128x128
                tile) + DVE copy out of PSUM. Emitted one pipeline step after
                embed_tile(b, j) so the PE never waits on the LN chain."""
                if stages < 2:
                    return
                t = seq[b]
                x = t[f"x{j}"]
                if XT_DMA:
                    nc.sync.dma_start_transpose(
                        t[f"xt{j // tpc}"][:, :, (j % tpc) * P:(j % tpc + 1) * P],
                        x[:])
                    return
                pst = ps_gv.tile([P, dk, P], F16, tag="gv", name="ps_t")
                for c in range(dk):
                    nc.tensor.transpose(
                        out=pst[:, c, :], in_=x[:, c * P:(c + 1) * P],
                        identity=ident[:])
                nc.vector.tensor_copy(
                    out=t[f"xt{j // tpc}"][:, :, (j % tpc) * P:(j % tpc + 1) * P],
                    in_=pst[:])

            def vproj(b, j, pool=None):
                if stages < 3:
                    return
                t = seq[b]
                xt = t[f"xt{j // tpc}"]
                jo = (j % tpc) * P
                ps = (ps_o.tile([P, D], F32, tag="o", name="ps_v")
                      if pool is ps_o else
                      ps_gv.tile([P, D], F32, tag="gv", name="ps_v"))
                for di in range(dk):
                    nc.tensor.matmul(
                        out=ps[:], lhsT=xt[:, di, jo:jo + P],
                        rhs=wv_t[:, di, :], start=(di == 0), stop=(di == dk - 1))
                if apply_beta:
                    nc.vector.tensor_tensor(
                        out=t["v"][:, j, :], in0=ps[:], in1=vb_t[:],
                        op=ALU.add)
                elif j % 2 == 0:
                    nc.vector.tensor_copy(out=t["v"][:, j, :], in_=ps[:])
                else:
                    nc.scalar.copy(out=t["v"][:, j, :], in_=ps[:])

            def gproj(b, ch, djs=None, eng=None):
                if stages < 3:
                    return
                t = seq[b]
                xt = t[f"xt{ch}"]
                for dj in (range(dk) if djs is None else djs):
                    ps = ps_gv.tile([P, ckw], F32, tag="gv", name="ps_g")
                    for di in range(dk):
                        nc.tensor.matmul(
                            out=ps[:], lhsT=a_t[:, di, dj * P:(dj + 1) * P],
                            rhs=xt[:, di, :],
                            start=(di == 0), stop=(di == dk - 1))
                    if apply_beta:
                        nc.scalar.activation(
                            out=t[f"gt{ch}"][:, dj, :], in_=ps[:],
                            func=AF.Identity,
                            bias=gb_t[:, dj:dj + 1], scale=1.0)
                    elif eng == "dve" or (eng is None and dj % 2 == 1):
                        nc.vector.tensor_copy(
                            out=t[f"gt{ch}"][:, dj, :], in_=ps[:])
                    else:
                        nc.scalar.copy(out=t[f"gt{ch}"][:, dj, :], in_=ps[:])

            def emit_dbg(b):
                if debug and b == 0 and stages >= 3:
                    t = seq[b]
                    fv = ebuf.tile([P, nt * D], F32, tag="dbg_v")
                    nc.vector.tensor_copy(
                        out=fv[:], in_=t["v"][:].rearrange("p a d -> p (a d)"))
                    nc.gpsimd.dma_start(out=dbg["v"][:, :], in_=fv[:])
                    fx = ebuf.tile([P, nt * D], F32, tag="dbg_x")
                    for jj in range(nt):
                        nc.vector.tensor_copy(
                            out=fx[:, jj * D:(jj + 1) * D], in_=t[f"x{jj}"][:])
                    nc.gpsimd.dma_start(out=dbg["x"][:, :], in_=fx[:])
                    for name in ("xt", "gt"):
                        f = ebuf.tile([P, dk, s_len], F32, tag=f"dbg_{name}")
                        for ch in range(nch):
                            nc.vector.tensor_copy(
                                out=f[:, :, ch * ckw:(ch + 1) * ckw],
                                in_=t[f"{name}{ch}"][:])
                        nc.gpsimd.dma_start(
                            out=dbg[name][:, :],
                            in_=f[:].rearrange("p a d -> p (a d)"))

            s_tiles = [None] * nt
            pt_tiles = [None] * nt

            def attn_a(b, j):
                if stages < 4:
                    return
                t = seq[b]
                gt = t[f"gt{j // tpc}"]
                jo = (j % tpc) * P
                s_ps = ps_s.tile([P, nch, ckw], F32, tag="s", name="ps_s_t")
                s_tiles[j] = s_ps
                for ch in range(nch):
                    for di in range(dk):
                        nc.tensor.matmul(
                            out=s_ps[:, ch, :],
                            lhsT=gt[:, di, jo:jo + P],
                            rhs=t[f"xt{ch}"][:, di, :],
                            start=(di == 0), stop=(di == dk - 1))

            def attn_b(b, j):
                if stages < 5:
                    return
                t = seq[b]
                s_ps = s_tiles[j]
                if debug and b == 0 and j == 0:
                    sf = ebuf.tile([P, s_len], F32, tag="dbg_s")
                    nc.vector.tensor_copy(
                        out=sf[:].rearrange("p (a d) -> p a d", a=nch),
                        in_=s_ps[:])
                    nc.gpsimd.dma_start(out=dbg["s"][:, :], in_=sf[:])
                nc.vector.tensor_reduce(
                    out=t[f"nm{j}"][:], in_=s_ps[:], axis=AX.XY,
                    op=ALU.max, negate=True)
                p16 = sm.tile([P, s_len], F16, tag="p16")
                nc.scalar.activation(
                    out=p16[:].rearrange("p (a d) -> p a d", a=nch),
                    in_=s_ps[:], func=AF.Exp,
                    bias=t[f"nm{j}"][:], scale=1.0,
                    accum_out=t[f"ls{j}"][:])
                nc.vector.reciprocal(
                    out=t[f"rr{j}"][:], in_=t[f"ls{j}"][:])
                pt16 = sm.tile([P, nt, P], F16, tag="pt16")
                pt_tiles[j] = pt16
                nc.sync.dma_start_transpose(pt16[:], p16[:])
                if debug and b == 0 and j == 0:
                    pf = ebuf.tile([P, s_len], F32, tag="dbg_p")
                    nc.vector.tensor_copy(out=pf[:], in_=p16[:])
                    nc.gpsimd.dma_start(out=dbg["p"][:, :], in_=pf[:])

            def attn_c(b, j):
                if stages < 6:
                    return
                t = seq[b]
                pt16 = pt_tiles[j]
                o_ps = ps_o.tile([P, D], F32, tag="o")
                for k in range(nt):
                    nc.tensor.matmul(
                        out=o_ps[:], lhsT=pt16[:, k, :], rhs=t["v"][:, k, :],
                        start=(k == 0), stop=(k == nt - 1))
                o16 = sm.tile([P, D], F16, tag="o16")
                if b == b_per_core - 1:
                    # last sequence: scale on the (idle-at-tail) DVE so the
                    # final outputs don't queue behind the ACT exp chain
                    nc.vector.tensor_scalar_mul(
                        out=o16[:], in0=o_ps[:], scalar1=t[f"rr{j}"][:])
                else:
                    nc.scalar.activation(
                        out=o16[:], in_=o_ps[:], func=AF.Identity,
                        bias=0.0, scale=t[f"rr{j}"][:])
                row = (b * nt + j) * P
                nc.sync.dma_start(out=out_d[row:row + P, :], in_=o16[:])

            def emit_attn(b):
                """Attention for seq b, with the embed chain of seq b+1
                emitted inside the loop and PV lagging S by 2 q-tiles; the
                last two PV stages interleave with seq b+1's projections so
                the PE never waits on the softmax->transpose chain."""
                nxt = b + 1 if b + 1 < b_per_core else None
                lag = min(7, nt - 1)
                spread = nxt is not None and nt == 8
                for j in range(nt):
                    if j >= lag:
                        attn_c(b, j - lag)
                    if nxt is not None:
                        embed_tile(nxt, j)
                        if j >= 1:
                            embed_xpose(nxt, j - 1)
                        if spread and j >= 2:
                            vproj(nxt, j - 2)
                    attn_a(b, j)
                    attn_b(b, j)
                    if spread and j >= 6:
                        gproj(nxt, 0, djs=[2 * (j - 6), 2 * (j - 6) + 1])
                tail = list(range(nt - lag, nt))
                if spread:
                    # interleave the pending PV stages with the remaining
                    # GV pieces of the next sequence
                    gv_tail = [
                        lambda: embed_xpose(nxt, nt - 1),
                        lambda: vproj(nxt, nt - 2),
                        lambda: vproj(nxt, nt - 1),
                    ] + [lambda dj=dj: gproj(nxt, 1, djs=[dj])
                         for dj in range(dk)]
                    gi = iter(gv_tail)
                    for j in tail:
                        attn_c(b, j)
                        piece = next(gi, None)
                        if piece is not None:
                            piece()
                    for piece in gi:
                        piece()
                elif nxt is not None:
                    attn_c(b, tail[0])
                    embed_xpose(nxt, nt - 1)
                    for j in range(min(4, nt)):
                        vproj(nxt, j)
                    for j in tail[1:-1]:
                        attn_c(b, j)
                    gproj(nxt, 0)
                    attn_c(b, tail[-1])
                    for j in range(4, nt):
                        vproj(nxt, j)
                    gproj(nxt, 1)
                else:
                    for j in tail:
                        attn_c(b, j)
                del s_tiles[:]
                s_tiles.extend([None] * nt)
                if b in seq and nxt is not None:
                    del seq[b]

            # ---------------- schedule ----------------
            if b_per_core > 0 and stages >= 1:
                if nt == 8:
                    # prologue: pipeline seq 0's GV into its embed chain; V
                    # uses the (otherwise idle) ps_o pool so V and G psum
                    # recycling don't serialize on each other.
                    embed_tile(0, 0)
                    embed_tile(0, 1)
                    late_loads()
                    emit_ident_warm()
                    embed_xpose(0, 0)
                    embed_tile(0, 2)
                    embed_xpose(0, 1)
                    embed_tile(0, 3)
                    embed_xpose(0, 2)
                    embed_tile(0, 4)
                    embed_xpose(0, 3)
                    vproj(0, 0, pool=ps_o)
                    vproj(0, 1, pool=ps_o)
                    embed_tile(0, 5)
                    embed_xpose(0, 4)
                    vproj(0, 2, pool=ps_o)
                    vproj(0, 3, pool=ps_o)
                    embed_tile(0, 6)
                    embed_xpose(0, 5)
                    gproj(0, 0, djs=[0, 1])
                    embed_tile(0, 7)
                    embed_xpose(0, 6)
                    gproj(0, 0, djs=[2, 3])
                    embed_xpose(0, 7)
                    for j in range(4, 8):
                        vproj(0, j, pool=ps_o)
                    gproj(0, 1)
                else:
                    for j in range(nt):
                        embed_tile(0, j)
                    late_loads()
                    emit_ident_warm()
                    for j in range(nt):
                        embed_xpose(0, j)
                    for j in range(min(4, nt)):
                        vproj(0, j)
                    gproj(0, 0)
                    for j in range(4, nt):
                        vproj(0, j)
                    gproj(0, 1)
                emit_dbg(0)
                for b in range(b_per_core):
                    emit_attn(b)

    fix_fat_waits(nc)
    return nc


_CACHE = {}


def _get_module(b_per_core, s_len, voc, apply_beta, stages=99):
    key = (b_per_core, s_len, voc, apply_beta, stages)
    if key not in _CACHE:
        _CACHE[key] = build(b_per_core, s_len, voc, apply_beta, stages=stages)
    return _CACHE[key]


def _host_fold(word_emb, pos_emb, gamma, beta, Wk, Wq, Wv, S):
    """Host-side weight folding: A = diag(g) Wq Wk^T diag(g)/sqrt(D), double
    centered (exact under zero-row-mean LN output); beta handled via rank-1
    bias terms on G and V."""
    g64 = gamma.astype(np.float64)
    b64 = beta.astype(np.float64)
    M = Wq.astype(np.float64) @ Wk.astype(np.float64).T / math.sqrt(D)
    A = g64[:, None] * M * g64[None, :]
    A = A - A.mean(0, keepdims=True)
    A = A - A.mean(1, keepdims=True)
    a16 = np.ascontiguousarray(A.astype(np.float16))

    wv16 = np.ascontiguousarray((g64[:, None] * Wv.astype(np.float64))
                                .astype(np.float16))

    apply_beta = bool(np.any(beta != 0.0))
    folds = {"amat": a16, "wv": wv16}
    if apply_beta:
        v = (b64 @ M) * g64
        v = v - v.mean()
        folds["gbias"] = np.ascontiguousarray(
            v.reshape(D // P, P).T.astype(np.float32))
        vb = (b64 @ Wv.astype(np.float64)).astype(np.float16)
        folds["vbias"] = np.ascontiguousarray(
            np.broadcast_to(vb, (P, D)).astype(np.float16))
    return folds, apply_beta


def kernel(input, word_emb, pos_emb, gamma, beta, Wk, Wq, Wv):
    input = np.asarray(input)
    word_emb = np.asarray(word_emb, dtype=np.float32)
    pos_emb = np.asarray(pos_emb, dtype=np.float32)
    gamma = np.asarray(gamma, dtype=np.float32)
    beta = np.asarray(beta, dtype=np.float32)
    Wk = np.asarray(Wk, dtype=np.float32)
    Wq = np.asarray(Wq, dtype=np.float32)
    Wv = np.asarray(Wv, dtype=np.float32)

    B, S = input.shape
    voc, d = word_emb.shape
    assert d == D
    b_per_core = B // N_CORES
    nt = S // P

    folds, apply_beta = _host_fold(word_emb, pos_emb, gamma, beta, Wk, Wq, Wv, S)
    wemb16 = np.ascontiguousarray(word_emb.astype(np.float16))
    pos16 = np.ascontiguousarray(pos_emb[:S].astype(np.float16))

    nc = _get_module(b_per_core, S, voc, apply_beta)

    ids32 = input.astype(np.int32)  # [B, S]
    in_maps = []
    for c in range(N_CORES):
        shard = ids32[c * b_per_core:(c + 1) * b_per_core]       # [bpc, S]
        ids_col = np.ascontiguousarray(
            shard.reshape(b_per_core * nt, P).T)                 # [128, bpc*nt]
        m = {"ids": ids_col, "wemb": wemb16, "pos": pos16, **folds}
        in_maps.append(m)

    res = run_bass_kernel_spmd(nc, in_maps, core_ids=list(range(N_CORES)))
    out = np.concatenate(
        [r["out"].reshape(b_per_core, S, D) for r in res.results],
        axis=0).astype(np.float32)
    return out


# revision 107
# speedup vs baseline: 2.3025x; 1.4900x over previous
"""MiniBert (embed + LayerNorm + single-head attention) on 8 TRN2 NeuronCores.

Strategy: data-parallel over batch (4 sequences per core); embedding table,
pos table and projection matrices replicated to every core.

Key algebraic restructuring vs the straightforward version:
  S = x Wq Wk^T x^T / sqrt(D) = x A x^T with A folded on the host
  (A = diag(g) Wq Wk^T diag(g) / sqrt(D)). Since LayerNorm output rows have
  zero mean, the rank-1 row/col means of A contribute only softmax-invariant
  row constants, so A is double-centered on the host; that both removes one
  of the two Q/K projections on device and makes A fp16-friendly (entries
  collapse from ~5.7 +- 0.4 to ~0 +- 0.4).

Per-core pipeline (per sequence, all fp16 operands, fp32 accumulation):
  gather word_emb rows (indirect DMA, fp16) -> += pos_emb (DVE)
  LayerNorm per token tile: bn_stats/bn_aggr (DVE), rsqrt via exp(-.5 ln) on
  ACT, normalize on ACT -> x fp16; x^T per token tile via PE fp16 transposes
  (53ns per 128x128) + DVE copy out of PSUM
  G^T = A^T-slices @ x^T; V = x^T-slices @ Wv'  (PE, fp16)
  S(j) = G^T.T @ x^T in PSUM fp32; row-max (DVE), P = exp(S-max) fp16 (ACT)
  with accumulated row sums; P^T via DMA transpose; O = P^T.T @ V in PSUM,
  scaled by 1/rowsum (ACT) -> fp16 out.
Scheduling: the whole kernel is software-pipelined for the in-order engine
queues. The embed/LN/transpose chain and the G/V projections of sequence b+1
are spread through the attention loop of sequence b; PV(j) trails S(j) by 7
q-tiles (all eight P^T tiles triple-buffered in SBUF) so the PE never waits
on the softmax -> DMA-transpose chain; PSUM pools are split S/GV/O so their
buffer recycling chains don't serialize; warm-up matmuls ahead of the first
gather ramp the PE clock during the initial DMA wait.
"""
import math
import numpy as np

from concourse import bass, mybir
import concourse.tile as tile
from concourse.bass_utils import run_bass_kernel_spmd
from concourse.masks import make_identity

P = 128
XT_DMA = False
D = 512
VOC = 32000
N_CORES = 8

F32 = mybir.dt.float32
F16 = mybir.dt.float16
I32 = mybir.dt.int32

AF = mybir.ActivationFunctionType
ALU = mybir.AluOpType
AX = mybir.AxisListType


def fix_fat_waits(nc, max_waits=1):
    """Walrus rejects instructions carrying more than ~1 semaphore wait. Tile
    occasionally emits joins (notably the kernel-tail drain) with one wait per
    producing processor. Split the extras into a chain of single-wait NoOps on
    the same engine, inserted immediately before the original instruction."""
    n_new = 0
    for bb in nc.main_func.blocks:
        insts = bb.instructions
        i = 0
        while i < len(insts):
            ins = insts[i]
            si = ins.sync_info
            if si and si.on_wait and len(si.on_wait) > max_waits:
                waits = list(si.on_wait)
                keep = waits[-max_waits:]
                extra = waits[:-max_waits]
                ins.sync_info = mybir.SyncInfo(
                    on_wait=keep, on_update=list(si.on_update or []))
                for j, w in enumerate(extra):
                    nop = mybir.InstNoOp(name=f"W-split-{n_new}", ins=[], outs=[])
                    n_new += 1
                    nop.engine = ins.engine
                    nop.sync_info = mybir.SyncInfo(on_wait=[w], on_update=[])
                    insts.insert(i + j, nop)
                i += len(extra)
            i += 1
    return n_new


def build(b_per_core: int, s_len: int, voc: int, apply_beta: bool,
          debug: bool = False, stages: int = 99, warm_mms: int = 8):
    """Build the per-core SPMD program. All cores run this same module."""
    nt = s_len // P          # token tiles per sequence
    dk = D // P              # feature tiles (4)
    nch = 2                  # free-dim chunks for the N=s_len matmuls
    ckw = s_len // nch

    nc = bass.Bass()

    dbg = {}
    if debug:
        dbg["x"] = nc.dram_tensor("dbg_x", [P, nt * D], F32, kind="ExternalOutput")
        dbg["xt"] = nc.dram_tensor("dbg_xt", [P, dk * s_len], F32, kind="ExternalOutput")
        dbg["gt"] = nc.dram_tensor("dbg_gt", [P, dk * s_len], F32, kind="ExternalOutput")
        dbg["v"] = nc.dram_tensor("dbg_v", [P, nt * D], F32, kind="ExternalOutput")
        dbg["s"] = nc.dram_tensor("dbg_s", [P, s_len], F32, kind="ExternalOutput")
        dbg["p"] = nc.dram_tensor("dbg_p", [P, s_len], F32, kind="ExternalOutput")

    ids_d = nc.dram_tensor("ids", [P, b_per_core * nt], I32, kind="ExternalInput")
    wemb_d = nc.dram_tensor("wemb", [voc, D], F16, kind="ExternalInput")
    pos_d = nc.dram_tensor("pos", [s_len, D], F16, kind="ExternalInput")
    a_d = nc.dram_tensor("amat", [D, D], F16, kind="ExternalInput")
    wv_d = nc.dram_tensor("wv", [D, D], F16, kind="ExternalInput")
    if apply_beta:
        gb_d = nc.dram_tensor("gbias", [P, dk], F32, kind="ExternalInput")
        vb_d = nc.dram_tensor("vbias", [P, D], F16, kind="ExternalInput")
    out_d = nc.dram_tensor("out", [b_per_core * s_len, D], F16, kind="ExternalOutput")

    with tile.TileContext(nc) as tc:
        with (
            tc.tile_pool(name="pers", bufs=1) as pers,
            tc.tile_pool(name="ebuf", bufs=3) as ebuf,
            tc.tile_pool(name="tbuf", bufs=2) as tbuf,
            tc.tile_pool(name="sm", bufs=8) as sm,
            tc.tile_pool(name="ps_s", bufs=2, space="PSUM") as ps_s,
            tc.tile_pool(name="ps_gv", bufs=2, space="PSUM") as ps_gv,
            tc.tile_pool(name="ps_o", bufs=2, space="PSUM") as ps_o,
        ):
            # ---------------- preamble: persistent tiles ----------------
            # The simulated DMA engine pool drains transfers roughly in
            # dispatch order, so keep the early-needed pieces small and first:
            # ids, then per-token-tile pos slices (sync queue) interleaving
            # with the first gathers; wv/a stream on the scalar queue.
            ids_t = pers.tile([P, b_per_core * nt], I32, tag="ids")
            nc.sync.dma_start(out=ids_t[:], in_=ids_d[:, :])

            pos_t = []
            for j in range(nt):
                pt_ = pers.tile([P, D], F16, tag=f"pos{j}", name=f"pos{j}_t")
                pos_t.append(pt_)
            wv_t = pers.tile([P, dk, D], F16, tag="wv")
            a_t = pers.tile([P, dk, D], F16, tag="a")
            npre = 6 if nt == 8 else nt
            for j in range(npre):
                nc.sync.dma_start(out=pos_t[j][:],
                                  in_=pos_d[j * P:(j + 1) * P, :])

            def late_loads():
                # deferred so the first gathers' transfers (critical path to
                # the first LayerNorm) aren't queued behind them on the DMA
                # engines.
                for j in range(npre, nt):
                    nc.sync.dma_start(out=pos_t[j][:],
                                      in_=pos_d[j * P:(j + 1) * P, :])
                nc.sync.dma_start(
                    out=wv_t[:], in_=wv_d.rearrange("(a p) n -> p a n", p=P))
                nc.sync.dma_start(
                    out=a_t[:], in_=a_d.rearrange("(a p) n -> p a n", p=P))

            if apply_beta:
                gb_t = pers.tile([P, dk], F32, tag="gb")
                nc.sync.dma_start(out=gb_t[:], in_=gb_d[:, :])
                vb_t = pers.tile([P, D], F16, tag="vb")
                nc.sync.dma_start(out=vb_t[:], in_=vb_d[:, :])

            epsb = pers.tile([P, 1], F32, tag="epsb")
            nc.vector.memset(epsb[:], 1e-5)
            # touch the ACT function table early so the (slow) table load
            # overlaps the preamble DMAs instead of the first LayerNorm.
            actw = pers.tile([P, 1], F32, tag="actw")
            nc.scalar.activation(out=actw[:], in_=epsb[:], func=AF.Exp,
                                 bias=0.0, scale=1.0)

            ident_f = pers.tile([P, P], F32, tag="ident_f")
            ident = pers.tile([P, P], F16, tag="ident")
            wscr = pers.tile([P, ckw], F16, tag="wscr")

            def emit_ident_warm():
                """PE-clock warm-up matmuls + identity for PE transposes.
                Emitted after the first gathers so make_identity's gpsimd
                work doesn't delay the first gather dispatch on the Pool
                queue. Warm-up results are never read."""
                if warm_mms > 0 and stages >= 3:
                    nc.vector.memset(wscr[:], 0.0)
                    for w in range(warm_mms):
                        wps = ps_gv.tile([P, ckw], F32, tag="gv")
                        nc.tensor.matmul(out=wps[:], lhsT=wscr[:, 0:P],
                                         rhs=wscr[:], start=True, stop=True)
                make_identity(nc, ident_f[:])
                nc.gpsimd.tensor_copy(out=ident[:], in_=ident_f[:])

            # ---------------- per-sequence emitters ----------------
            seq = {}   # per-seq tiles, rotated via pools

            tpc = nt // nch   # token tiles per chunk

            def embed_tile(b, j):
                """Gather+LN+transpose chain for token tile j of sequence b."""
                if stages < 1:
                    return
                if j == 0:
                    shapes = {"v": (tbuf, [P, nt, D], F16)}
                    for jj in range(nt):
                        shapes[f"e{jj}"] = (ebuf, [P, D], F16)
                        shapes[f"x{jj}"] = (ebuf, [P, D], F16)
                        shapes[f"s6_{jj}"] = (ebuf, [P, 6], F32)
                        shapes[f"mv{jj}"] = (ebuf, [P, 2], F32)
                        shapes[f"lnv{jj}"] = (ebuf, [P, 1], F32)
                        shapes[f"rs{jj}"] = (ebuf, [P, 1], F32)
                        shapes[f"nmu{jj}"] = (ebuf, [P, 1], F32)
                        shapes[f"m2_{jj}"] = (ebuf, [P, 1], F32)
                        shapes[f"nm{jj}"] = (ebuf, [P, 1], F32)
                        shapes[f"ls{jj}"] = (ebuf, [P, 1], F32)
                        shapes[f"rr{jj}"] = (ebuf, [P, 1], F32)
                    for ch in range(nch):
                        shapes[f"xt{ch}"] = (tbuf, [P, dk, ckw], F16)
                        shapes[f"gt{ch}"] = (tbuf, [P, dk, ckw], F16)
                    seq[b] = {k: pool.tile(shp, dt, tag=k, name=f"{k}_t")
                              for k, (pool, shp, dt) in shapes.items()}
                t = seq[b]
                e, x = t[f"e{j}"], t[f"x{j}"]
                nc.gpsimd.indirect_dma_start(
                    out=e[:], out_offset=None, in_=wemb_d[:],
                    in_offset=bass.IndirectOffsetOnAxis(
                        ap=ids_t[:, b * nt + j: b * nt + j + 1], axis=0))
                if stages < 2:
                    return
                mv, rs, nmu = t[f"mv{j}"], t[f"rs{j}"], t[f"nmu{j}"]
                nc.vector.tensor_tensor(
                    out=e[:], in0=e[:], in1=pos_t[j][:], op=ALU.add)
                nc.vector.bn_stats(out=t[f"s6_{j}"][:], in_=e[:])
                nc.vector.bn_aggr(out=mv[:], in_=t[f"s6_{j}"][:])
                # rs = exp(-0.5*ln(var+eps)) == rsqrt(var+eps)
                nc.scalar.activation(
                    out=t[f"lnv{j}"][:], in_=mv[:, 1:2],
                    func=AF.Ln, bias=epsb[:, 0:1], scale=1.0)
                nc.scalar.activation(
                    out=rs[:], in_=t[f"lnv{j}"][:],
                    func=AF.Exp, bias=0.0, scale=-0.5)
                # m2 = -mean on DVE (parallel with Ln/Exp above), then
                # nmu = m2*rs on ACT: keeps the serial chain on one engine
                m2 = t[f"m2_{j}"]
                nc.vector.tensor_scalar_mul(
                    out=m2[:], in0=mv[:, 0:1], scalar1=-1.0)
                nc.scalar.activation(
                    out=nmu[:], in_=rs[:], func=AF.Identity,
                    bias=0.0, scale=m2[:])
                nc.scalar.activation(
                    out=x[:], in_=e[:], func=AF.Identity,
                    bias=nmu[:], scale=rs[:])

            def embed_xpose(b, j):
                """x^T for tile j on the PE (fp16 transpose, 53ns per 128x128
                tile) + DVE copy out of PSUM. Emitted one pipeline step after
                embed_tile(b, j) so the PE never waits on the LN chain."""
                if stages < 2:
                    return
                t = seq[b]
                x = t[f"x{j}"]
                if XT_DMA:
                    nc.sync.dma_start_transpose(
                        t[f"xt{j // tpc}"][:, :, (j % tpc) * P:(j % tpc + 1) * P],
                        x[:])
                    return
                pst = ps_gv.tile([P, dk, P], F16, tag="gv", name="ps_t")
                for c in range(dk):
                    nc.tensor.transpose(
                        out=pst[:, c, :], in_=x[:, c * P:(c + 1) * P],
                        identity=ident[:])
                dst = t[f"xt{j // tpc}"][:, :, (j % tpc) * P:(j % tpc + 1) * P]
                if j % 2 == 0:
                    nc.vector.tensor_copy(out=dst, in_=pst[:])
                else:
                    nc.scalar.copy(out=dst, in_=pst[:])

            def vproj(b, j, pool=None):
                if stages < 3:
                    return
                t = seq[b]
                xt = t[f"xt{j // tpc}"]
                jo = (j % tpc) * P
                ps = (ps_o.tile([P, D], F32, tag="o", name="ps_v")
                      if pool is ps_o else
                      ps_gv.tile([P, D], F32, tag="gv", name="ps_v"))
                for di in range(dk):
                    nc.tensor.matmul(
                        out=ps[:], lhsT=xt[:, di, jo:jo + P],
                        rhs=wv_t[:, di, :], start=(di == 0), stop=(di == dk - 1))
                if apply_beta:
                    nc.vector.tensor_tensor(
                        out=t["v"][:, j, :], in0=ps[:], in1=vb_t[:],
                        op=ALU.add)
                elif j % 2 == 0:
                    nc.vector.tensor_copy(out=t["v"][:, j, :], in_=ps[:])
                else:
                    nc.scalar.copy(out=t["v"][:, j, :], in_=ps[:])

            def gproj(b, ch, djs=None, eng=None):
                if stages < 3:
                    return
                t = seq[b]
                xt = t[f"xt{ch}"]
                for dj in (range(dk) if djs is None else djs):
                    ps = ps_gv.tile([P, ckw], F32, tag="gv", name="ps_g")
                    for di in range(dk):
                        nc.tensor.matmul(
                            out=ps[:], lhsT=a_t[:, di, dj * P:(dj + 1) * P],
                            rhs=xt[:, di, :],
                            start=(di == 0), stop=(di == dk - 1))
                    if apply_beta:
                        nc.scalar.activation(
                            out=t[f"gt{ch}"][:, dj, :], in_=ps[:],
                            func=AF.Identity,
                            bias=gb_t[:, dj:dj + 1], scale=1.0)
                    elif eng == "dve" or (eng is None and dj % 2 == 1):
                        nc.vector.tensor_copy(
                            out=t[f"gt{ch}"][:, dj, :], in_=ps[:])
                    else:
                        nc.scalar.copy(out=t[f"gt{ch}"][:, dj, :], in_=ps[:])

            def emit_dbg(b):
                if debug and b == 0 and stages >= 3:
                    t = seq[b]
                    fv = ebuf.tile([P, nt * D], F32, tag="dbg_v")
                    nc.vector.tensor_copy(
                        out=fv[:], in_=t["v"][:].rearrange("p a d -> p (a d)"))
                    nc.gpsimd.dma_start(out=dbg["v"][:, :], in_=fv[:])
                    fx = ebuf.tile([P, nt * D], F32, tag="dbg_x")
                    for jj in range(nt):
                        nc.vector.tensor_copy(
                            out=fx[:, jj * D:(jj + 1) * D], in_=t[f"x{jj}"][:])
                    nc.gpsimd.dma_start(out=dbg["x"][:, :], in_=fx[:])
                    for name in ("xt", "gt"):
                        f = ebuf.tile([P, dk, s_len], F32, tag=f"dbg_{name}")
                        for ch in range(nch):
                            nc.vector.tensor_copy(
                                out=f[:, :, ch * ckw:(ch + 1) * ckw],
                                in_=t[f"{name}{ch}"][:])
                        nc.gpsimd.dma_start(
                            out=dbg[name][:, :],
                            in_=f[:].rearrange("p a d -> p (a d)"))

            s_tiles = [None] * nt
            pt_tiles = [None] * nt

            def attn_a(b, j):
                if stages < 4:
                    return
                t = seq[b]
                gt = t[f"gt{j // tpc}"]
                jo = (j % tpc) * P
                s_ps = ps_s.tile([P, nch, ckw], F32, tag="s", name="ps_s_t")
                s_tiles[j] = s_ps
                for ch in range(nch):
                    for di in range(dk):
                        nc.tensor.matmul(
                            out=s_ps[:, ch, :],
                            lhsT=gt[:, di, jo:jo + P],
                            rhs=t[f"xt{ch}"][:, di, :],
                            start=(di == 0), stop=(di == dk - 1))

            def attn_b(b, j):
                if stages < 5:
                    return
                t = seq[b]
                s_ps = s_tiles[j]
                if debug and b == 0 and j == 0:
                    sf = ebuf.tile([P, s_len], F32, tag="dbg_s")
                    nc.vector.tensor_copy(
                        out=sf[:].rearrange("p (a d) -> p a d", a=nch),
                        in_=s_ps[:])
                    nc.gpsimd.dma_start(out=dbg["s"][:, :], in_=sf[:])
                nc.vector.tensor_reduce(
                    out=t[f"nm{j}"][:], in_=s_ps[:], axis=AX.XY,
                    op=ALU.max, negate=True)
                p16 = sm.tile([P, s_len], F16, tag="p16")
                nc.scalar.activation(
                    out=p16[:].rearrange("p (a d) -> p a d", a=nch),
                    in_=s_ps[:], func=AF.Exp,
                    bias=t[f"nm{j}"][:], scale=1.0,
                    accum_out=t[f"ls{j}"][:])
                nc.vector.reciprocal(
                    out=t[f"rr{j}"][:], in_=t[f"ls{j}"][:])
                pt16 = sm.tile([P, nt, P], F16, tag="pt16")
                pt_tiles[j] = pt16
                nc.sync.dma_start_transpose(pt16[:], p16[:])
                if debug and b == 0 and j == 0:
                    pf = ebuf.tile([P, s_len], F32, tag="dbg_p")
                    nc.vector.tensor_copy(out=pf[:], in_=p16[:])
                    nc.gpsimd.dma_start(out=dbg["p"][:, :], in_=pf[:])

            def attn_c(b, j):
                if stages < 6:
                    return
                t = seq[b]
                pt16 = pt_tiles[j]
                o_ps = ps_o.tile([P, D], F32, tag="o")
                for k in range(nt):
                    nc.tensor.matmul(
                        out=o_ps[:], lhsT=pt16[:, k, :], rhs=t["v"][:, k, :],
                        start=(k == 0), stop=(k == nt - 1))
                o16 = sm.tile([P, D], F16, tag="o16")
                if b == b_per_core - 1:
                    # last sequence: scale on the (idle-at-tail) DVE so the
                    # final outputs don't queue behind the ACT exp chain
                    nc.vector.tensor_scalar_mul(
                        out=o16[:], in0=o_ps[:], scalar1=t[f"rr{j}"][:])
                else:
                    nc.scalar.activation(
                        out=o16[:], in_=o_ps[:], func=AF.Identity,
                        bias=0.0, scale=t[f"rr{j}"][:])
                row = (b * nt + j) * P
                nc.sync.dma_start(out=out_d[row:row + P, :], in_=o16[:])

            def emit_attn(b):
                """Attention for seq b, with the embed chain of seq b+1
                emitted inside the loop and PV lagging S by 2 q-tiles; the
                last two PV stages interleave with seq b+1's projections so
                the PE never waits on the softmax->transpose chain."""
                nxt = b + 1 if b + 1 < b_per_core else None
                lag = min(7, nt - 1)
                spread = nxt is not None and nt == 8
                for j in range(nt):
                    if j >= lag:
                        attn_c(b, j - lag)
                    if nxt is not None:
                        embed_tile(nxt, j)
                        if j >= 1:
                            embed_xpose(nxt, j - 1)
                        if spread and j >= 2:
                            vproj(nxt, j - 2)
                    attn_a(b, j)
                    attn_b(b, j)
                    if spread and j >= 6:
                        gproj(nxt, 0, djs=[2 * (j - 6), 2 * (j - 6) + 1])
                tail = list(range(nt - lag, nt))
                if spread:
                    # interleave the pending PV stages with the remaining
                    # GV pieces of the next sequence
                    gv_tail = [
                        lambda: embed_xpose(nxt, nt - 1),
                        lambda: vproj(nxt, nt - 2),
                        lambda: vproj(nxt, nt - 1),
                    ] + [lambda dj=dj: gproj(nxt, 1, djs=[dj])
                         for dj in range(dk)]
                    gi = iter(gv_tail)
                    for j in tail:
                        attn_c(b, j)
                        piece = next(gi, None)
                        if piece is not None:
                            piece()
                    for piece in gi:
                        piece()
                elif nxt is not None:
                    attn_c(b, tail[0])
                    embed_xpose(nxt, nt - 1)
                    for j in range(min(4, nt)):
                        vproj(nxt, j)
                    for j in tail[1:-1]:
                        attn_c(b, j)
                    gproj(nxt, 0)
                    attn_c(b, tail[-1])
                    for j in range(4, nt):
                        vproj(nxt, j)
                    gproj(nxt, 1)
                else:
                    for j in tail:
                        attn_c(b, j)
                del s_tiles[:]
                s_tiles.extend([None] * nt)
                if b in seq and nxt is not None:
                    del seq[b]

            # ---------------- schedule ----------------
            if b_per_core > 0 and stages >= 1:
                if nt == 8:
                    # prologue: pipeline seq 0's GV into its embed chain; V
                    # uses the (otherwise idle) ps_o pool so V and G psum
                    # recycling don't serialize on each other.
                    embed_tile(0, 0)
                    embed_tile(0, 1)
                    late_loads()
                    emit_ident_warm()
                    embed_xpose(0, 0)
                    embed_tile(0, 2)
                    embed_xpose(0, 1)
                    embed_tile(0, 3)
                    embed_xpose(0, 2)
                    embed_tile(0, 4)
                    embed_xpose(0, 3)
                    vproj(0, 0, pool=ps_o)
                    vproj(0, 1, pool=ps_o)
                    embed_tile(0, 5)
                    embed_xpose(0, 4)
                    vproj(0, 2, pool=ps_o)
                    vproj(0, 3, pool=ps_o)
                    embed_tile(0, 6)
                    embed_xpose(0, 5)
                    gproj(0, 0, djs=[0, 1])
                    embed_tile(0, 7)
                    embed_xpose(0, 6)
                    gproj(0, 0, djs=[2, 3])
                    embed_xpose(0, 7)
                    for j in range(4, 8):
                        vproj(0, j, pool=ps_o)
                    gproj(0, 1)
                else:
                    for j in range(nt):
                        embed_tile(0, j)
                    late_loads()
                    emit_ident_warm()
                    for j in range(nt):
                        embed_xpose(0, j)
                    for j in range(min(4, nt)):
                        vproj(0, j)
                    gproj(0, 0)
                    for j in range(4, nt):
                        vproj(0, j)
                    gproj(0, 1)
                emit_dbg(0)
                for b in range(b_per_core):
                    emit_attn(b)

    fix_fat_waits(nc)
    return nc


_CACHE = {}


def _get_module(b_per_core, s_len, voc, apply_beta, stages=99):
    key = (b_per_core, s_len, voc, apply_beta, stages)
    if key not in _CACHE:
        _CACHE[key] = build(b_per_core, s_len, voc, apply_beta, stages=stages)
    return _CACHE[key]


def _host_fold(word_emb, pos_emb, gamma, beta, Wk, Wq, Wv, S):
    """Host-side weight folding: A = diag(g) Wq Wk^T diag(g)/sqrt(D), double
    centered (exact under zero-row-mean LN output); beta handled via rank-1
    bias terms on G and V."""
    g64 = gamma.astype(np.float64)
    b64 = beta.astype(np.float64)
    M = Wq.astype(np.float64) @ Wk.astype(np.float64).T / math.sqrt(D)
    A = g64[:, None] * M * g64[None, :]
    A = A - A.mean(0, keepdims=True)
    A = A - A.mean(1, keepdims=True)
    a16 = np.ascontiguousarray(A.astype(np.float16))

    wv16 = np.ascontiguousarray((g64[:, None] * Wv.astype(np.float64))
                                .astype(np.float16))

    apply_beta = bool(np.any(beta != 0.0))
    folds = {"amat": a16, "wv": wv16}
    if apply_beta:
        v = (b64 @ M) * g64
        v = v - v.mean()
        folds["gbias"] = np.ascontiguousarray(
            v.reshape(D // P, P).T.astype(np.float32))
        vb = (b64 @ Wv.astype(np.float64)).astype(np.float16)
        folds["vbias"] = np.ascontiguousarray(
            np.broadcast_to(vb, (P, D)).astype(np.float16))
    return folds, apply_beta


def kernel(input, word_emb, pos_emb, gamma, beta, Wk, Wq, Wv):
    input = np.asarray(input)
    word_emb = np.asarray(word_emb, dtype=np.float32)
    pos_emb = np.asarray(pos_emb, dtype=np.float32)
    gamma = np.asarray(gamma, dtype=np.float32)
    beta = np.asarray(beta, dtype=np.float32)
    Wk = np.asarray(Wk, dtype=np.float32)
    Wq = np.asarray(Wq, dtype=np.float32)
    Wv = np.asarray(Wv, dtype=np.float32)

    B, S = input.shape
    voc, d = word_emb.shape
    assert d == D
    b_per_core = B // N_CORES
    nt = S // P

    folds, apply_beta = _host_fold(word_emb, pos_emb, gamma, beta, Wk, Wq, Wv, S)
    wemb16 = np.ascontiguousarray(word_emb.astype(np.float16))
    pos16 = np.ascontiguousarray(pos_emb[:S].astype(np.float16))

    nc = _get_module(b_per_core, S, voc, apply_beta)

    ids32 = input.astype(np.int32)  # [B, S]
    in_maps = []
    for c in range(N_CORES):
        shard = ids32[c * b_per_core:(c + 1) * b_per_core]       # [bpc, S]
        ids_col = np.ascontiguousarray(
            shard.reshape(b_per_core * nt, P).T)                 # [128, bpc*nt]
        m = {"ids": ids_col, "wemb": wemb16, "pos": pos16, **folds}
        in_maps.append(m)

    res = run_bass_kernel_spmd(nc, in_maps, core_ids=list(range(N_CORES)))
    out = np.concatenate(
        [r["out"].reshape(b_per_core, S, D) for r in res.results],
        axis=0).astype(np.float32)
    return out
